# revision 21
# baseline (speedup 1.0000x reference)
"""Trainium2 Bass kernel for HandmadeConv2d.

Conv2d NCHW, valid padding, stride 1, no bias:
  x: (32, 128, 64, 64) f32, weights: (256, 128, 3, 3) f32 -> out: (32, 256, 62, 62) f32

Sharding: data-parallel over batch, 4 images per core across 8 NeuronCores;
weights replicated.

Per core the conv is computed as 9 accumulating matmuls per output tile:
  out[oc, (oh,ow)] += W[kh,kw][ic, oc].T @ x[ic, (oh+kh, ow+kw)]
with ic=128 as the PE contraction dim, oc split into 2 chunks of 128
(PSUM partition dim), and spatial tiled as 8 output rows x 62 cols = 496
moving-operand elements (<=512 fp32 limit, fits one PSUM bank).

Matmul dtype is bf16 (single pass): measured on TRN2, bf16 matmuls run
within ~2% of the 1 cycle/row PE roofline while fp32r pays a fixed ~48
cycle per-instruction penalty plus ~13 cycles per moving-AP row jump
(~13% total). Single-pass bf16 rounding gives ~2.4e-3 relative error on
this 1152-term contraction, well inside the 2e-2 gate. Accumulation is
fp32 in PSUM.

All data preparation happens on the host: weights are pre-transposed to
[ic, kh*kw, oc] and cast to bf16, so the device performs zero weight
transposes and zero dtype casts.
"""

import os
import warnings

warnings.filterwarnings("ignore")

import numpy as np

N_CORES = 8
NIMG = 4  # images per core
IC = 128
OC = 256
H = W = 64
OH = OW = 62
P = 128

MODE = os.environ.get("BASS_CONV_MODE", "hyb")

_NC_CACHE = {}

# x row-bands (2-row halo) so first matmuls start after ~1/4 image is resident
BANDS = [(0, 18), (16, 18), (32, 18), (48, 16)]  # (row0, nrows)

# PE pre-warm count: dummy matmuls bridge the initial DMA wait so the PE
# p-state ramps (0.65 -> 2.4 GHz) before real work, and the PE never goes
# idle at the warm->real handoff (an idle gap re-throttles the clock and
# costs ~4us of half-speed matmuls).
N_WARM = 30


def _row_groups():
    groups = []
    r = 0
    while r < OH:
        nr = min(8, OH - r)
        groups.append((r, nr))
        r += nr
    return groups


def round_fp32r(a):
    """Round fp32 to the PE's fp32r format: RNE keeping 11 mantissa bits."""
    u = np.ascontiguousarray(a, dtype=np.float32).view(np.uint32)
    low = u & np.uint32(0xFFF)
    base = u & np.uint32(0xFFFFF000)
    lsb = (u >> np.uint32(12)) & np.uint32(1)
    up = (low > 0x800) | ((low == 0x800) & (lsb == 1))
    r = base + (up.astype(np.uint32) << np.uint32(12))
    return r.view(np.float32).reshape(a.shape)


def build_nc(mode):
    import concourse.bacc as bacc
    import concourse.mybir as mybir
    import concourse.tile as tile

    f32 = mybir.dt.float32
    f8 = mybir.dt.float8e4
    hyb = mode == "hyb"
    if mode in ("bf16", "hyb"):
        ddt = mybir.dt.bfloat16
    elif mode == "fp32r":
        ddt = mybir.dt.float32r
    else:
        raise ValueError(mode)

    nc = bacc.Bacc("TRN2", target_bir_lowering=False, debug=False)
    xh = nc.dram_tensor("xh", [NIMG, IC, H, W], ddt, kind="ExternalInput")
    wh = nc.dram_tensor("wh", [IC, 9, OC], ddt, kind="ExternalInput")
    if hyb:
        # fp8 copies for the two kw=0 taps computed in DoubleRow mode.
        # Weights carry a 64x scale (pushed into the bf16 weights too) so
        # fp8 weight values clear e4m3's denormal range; the PSUM->SBUF
        # copy divides by 64.
        x8h = nc.dram_tensor("x8h", [NIMG, IC, H, W], f8, kind="ExternalInput")
        w8h = nc.dram_tensor("w8h", [IC, 2, OC], f8, kind="ExternalInput")
    out = nc.dram_tensor("out", [NIMG, OC, OH, OW], f32, kind="ExternalOutput")

    groups = _row_groups()

    with tile.TileContext(nc) as tc:
        with (
            tc.tile_pool(name="wtiles", bufs=1) as wtiles,
            tc.tile_pool(name="xconv", bufs=8) as xconv,
            tc.tile_pool(name="osb", bufs=8) as osb,
            tc.tile_pool(name="psmm", bufs=8, space="PSUM") as psmm,
        ):
            wt = wtiles.tile([P, 9, OC], ddt, tag="wt")
            warm = wtiles.tile([P, 256], mybir.dt.bfloat16, tag="warm")

            # Startup-ordered DMAs across three queues, sequenced by when the
            # first group's 9-matmul chain needs each piece: image-0 band 0
            # leads the Sync queue, weight k-slices 0..5 lead the Scalar
            # queue, k6..8 + image-0 bands 1-3 ride GpSimd. All 64 output
            # DMAs go on Sync (HWDGE) -- routing them through GpSimd's SWDGE
            # makes the end-of-kernel drain ~4us slower.
            if hyb:
                w8t = wtiles.tile([P, 2, OC], f8, tag="w8t")

            def load_f8(n, tiles, engine):
                for bi, (b0, bn) in enumerate(BANDS):
                    xf8 = xconv.tile([P, 2, 18, W], f8, tag="xf8", name="xf8")
                    engine.dma_start(
                        xf8[:, 0, :bn, :], x8h[:][n, :, b0 : b0 + bn, :]
                    )
                    bn1 = min(bn, H - 1 - b0)
                    engine.dma_start(
                        xf8[:, 1, :bn1, :], x8h[:][n, :, b0 + 1 : b0 + 1 + bn1, :]
                    )
                    tiles[bi] = (tiles[bi], xf8)

            def load_bands(n, engine):
                tiles = []
                for b0, bn in BANDS:
                    xb = xconv.tile([P, 18, W], ddt, tag="xb", name="xb")
                    engine.dma_start(xb[:, :bn, :], xh[:][n, :, b0 : b0 + bn, :])
                    tiles.append(xb)
                if hyb:
                    load_f8(n, tiles, nc.gpsimd)
                return tiles

            xb0_tiles = [
                xconv.tile([P, 18, W], ddt, tag="xb", name="xb") for _ in BANDS
            ]
            b0, bn = BANDS[0]
            nc.sync.dma_start(xb0_tiles[0][:, :bn, :], xh[:][0, :, b0 : b0 + bn, :])
            nc.scalar.dma_start(wt[:], wh[:])
            if hyb:
                nc.scalar.dma_start(w8t[:], w8h[:])
            nc.gpsimd.memset(warm[:], 0.0)
            for bi in range(1, len(BANDS)):
                b0, bn = BANDS[bi]
                nc.scalar.dma_start(
                    xb0_tiles[bi][:, :bn, :], xh[:][0, :, b0 : b0 + bn, :]
                )
            if hyb:
                load_f8(0, xb0_tiles, nc.gpsimd)
            for _ in range(N_WARM):
                wps = psmm.tile([P, 512], f32, tag="mm", name="wps")
                nc.tensor.matmul(
                    wps[:, :256], warm[:, :P], warm[:, :256], start=True, stop=True
                )

            for n in range(NIMG):
                xb_tiles = xb0_tiles if n == 0 else load_bands(n, nc.scalar)

                for r0, nr in groups:
                    b = min(3, r0 // 16)
                    b0 = BANDS[b][0]
                    xt = xb_tiles[b]
                    if hyb:
                        xt, xf8 = xt
                    rloc = r0 - b0
                    bf16_ks = (1, 2, 4, 5, 6, 7, 8) if hyb else tuple(range(9))
                    for c in range(2):
                        ps_t = psmm.tile([P, 512], f32, tag="mm", name="ps_t")
                        for ki, k in enumerate(bf16_ks):
                            kh, kw = divmod(k, 3)
                            rr = rloc + kh
                            nc.tensor.matmul(
                                ps_t[:, : nr * OW],
                                wt[:, k, c * P : (c + 1) * P],
                                xt[:, rr : rr + nr, kw : kw + OW],
                                start=(ki == 0),
                                stop=(not hyb and ki == 8),
                                skip_group_check=hyb,
                            )
                        if hyb:
                            # taps (kh=0,kw=0)+(kh=1,kw=0) in one fp8
                            # DoubleRow pass, split into two matmuls to
                            # respect the 512 moving-free-dim limit.
                            h1 = nr // 2
                            for si, (ra, rb) in enumerate(((0, h1), (h1, nr))):
                                nc.tensor.matmul(
                                    ps_t[:, ra * OW : rb * OW],
                                    w8t[:, :, c * P : (c + 1) * P],
                                    xf8[:, :, rloc + ra : rloc + rb, 0:OW],
                                    start=False,
                                    stop=True,
                                    perf_mode=mybir.MatmulPerfMode.DoubleRow,
                                    skip_group_check=True,
                                )
                        ob = osb.tile([P, 8 * OW], f32, tag="ob", name="ob")
                        if hyb:
                            nc.vector.tensor_scalar_mul(
                                ob[:, : nr * OW], ps_t[:, : nr * OW], 1.0 / 64.0
                            )
                        else:
                            nc.any.tensor_copy(ob[:, : nr * OW], ps_t[:, : nr * OW])
                        nc.sync.dma_start(
                            out[:][n, c * P : (c + 1) * P, r0 : r0 + nr, :],
                            ob[:, : nr * OW].rearrange("p (r q) -> p r q", q=OW),
                        )

    nc.compile()
    return nc


def get_nc(mode=None):
    mode = mode or MODE
    if mode not in _NC_CACHE:
        _NC_CACHE[mode] = build_nc(mode)
    return _NC_CACHE[mode]


def _host_prep(x, weights, mode):
    """Host-side data prep: weight transpose to [ic, kh*kw, oc] plus
    per-mode rounding/casting."""
    x = np.ascontiguousarray(np.asarray(x), dtype=np.float32).reshape(-1, IC, H, W)
    w = np.ascontiguousarray(np.asarray(weights), dtype=np.float32)
    wt = np.ascontiguousarray(w.transpose(1, 2, 3, 0)).reshape(IC, 9, OC)

    if mode == "bf16":
        import ml_dtypes

        bf = ml_dtypes.bfloat16
        return {"xh": x.astype(bf), "wh": wt.astype(bf)}
    if mode == "hyb":
        import ml_dtypes

        bf = ml_dtypes.bfloat16
        f8 = ml_dtypes.float8_e4m3
        # taps k=0 (kh0,kw0) and k=3 (kh1,kw0) run in fp8 DoubleRow; all
        # weights carry a 64x scale (undone by the scaled PSUM copy).
        return {
            "xh": x.astype(bf),
            "wh": (wt * 64.0).astype(bf),
            "x8h": x.astype(f8),
            "w8h": np.ascontiguousarray((wt[:, (0, 3), :] * 64.0)).astype(f8),
        }
    if mode == "fp32r":
        return {"xh": round_fp32r(x), "wh": round_fp32r(wt)}
    raise ValueError(mode)


def kernel(x, weights, _trace=False, _mode=None):
    from concourse.bass_utils import run_bass_kernel_spmd

    mode = _mode or MODE
    nc = get_nc(mode)
    tensors = _host_prep(x, weights, mode)
    in_maps = []
    for i in range(N_CORES):
        m = {}
        for k, v in tensors.items():
            m[k] = v[i * NIMG : (i + 1) * NIMG] if k.startswith("x") else v
        in_maps.append(m)
    res = run_bass_kernel_spmd(
        nc, in_maps, core_ids=list(range(N_CORES)), trace=_trace
    )
    out = np.concatenate([r["out"] for r in res.results], axis=0)
    if _trace:
        kernel.last_results = res
    return out


kernel.last_results = None
